# revision 15
# baseline (speedup 1.0000x reference)
"""Gemma3 sliding-window attention layer on 8 Trainium2 NeuronCores.

Tensor-parallel over query heads: core h computes query head h (kv head
h//2), i.e. column-parallel qkv projection, full per-head attention, and
the row-parallel slice of o_proj; the 8 partial [S, H] bf16 outputs are
summed on the host in f32 (the all-reduce / unshard step).

Layout: hidden_states transposed on the host ([H, S]) so q and k come
out of the projection directly in [d, seq] layout (what the scores
matmul needs) and v comes out natural [seq, d] (what attn@v needs).
The softmax denominator rides as a ones-column appended to v.

Phase structure (interleaved per 512-token column group so Vector work
hides under the projection matmuls):
  per group g: qkv projection -> squares -> stats -> rope q/k
  per row r:   scores (+mask via PE accumulation) -> exp (k-rstd folded
               into the activation scale) -> attn@v -> transpose ->
               o_proj (softmax denom folded into the PSUM-drain scale)

RMSNorm: q's reciprocal-RMS is produced broadcast on all partitions by
a ones-matmul (with SCALING and the (1+w) gain folded into rope tables
/ eps); k's is produced per-key-partition (32 tiny matmuls) and applied
as the per-partition `scale` operand of the Exp activation, so k needs
no rstd multiply and no wide reciprocal at all.
"""

import numpy as np
import ml_dtypes

import concourse.bass as bass
import concourse.mybir as mybir
import concourse.tile as tile
from concourse.bass_utils import run_bass_kernel_spmd
from concourse.masks import make_identity

# ---- problem constants (hardcoded; kernel.py must be self-contained) ----
S = 2048          # sequence length
H = 2560          # hidden size
NH = 8            # query heads
NKV = 4           # kv heads
D = 256           # head dim
EPS = 1e-6
SCALING = 256.0 ** -0.5
WINDOW = 1024 - 1  # sliding window - 1

N_CORES = 8
KC = H // 128      # 20 contraction chunks for the projection
RB = S // 128      # 16 row blocks
GW = 512           # column group width
NG = S // GW       # 4 groups
MASK_VAL = -1e10
BLK_WIN = WINDOW // 128 + 1   # 8: c in [r-8, r] can contribute

F32 = mybir.dt.float32
BF16 = mybir.dt.bfloat16


def _bf16(x):
    return np.ascontiguousarray(x.astype(ml_dtypes.bfloat16))


def _chunk_part(x, p=128):
    """[c*p, n] -> [p, c, n] host relayout so it DMAs 1:1 into an SBUF tile."""
    c = x.shape[0] // p
    return np.ascontiguousarray(
        x.reshape(c, p, *x.shape[1:]).transpose(1, 0, *range(2, x.ndim + 1))
    )


def split_multiwaits(nc):
    """This toolchain's codegen allows one sync-wait slot per instruction.

    Tile emits several waits on the first consumer of multi-queue DMAs and
    on kernel-tail drains; hoist all but the last wait onto same-engine
    NoOps inserted immediately before the offending instruction (queue
    order on the engine preserves the semantics exactly).
    """
    k = 0
    for f in nc.m.functions:
        for bb in f.blocks:
            insts = bb.instructions
            if not any(i.sync_info and len(i.sync_info.on_wait) > 1
                       for i in insts):
                continue
            newlist = []
            for inst in insts:
                si = inst.sync_info
                if si is not None and len(si.on_wait) > 1:
                    for w in list(si.on_wait)[:-1]:
                        nop = mybir.InstNoOp(name=f"{inst.name}-ws{k}")
                        k += 1
                        nop.engine = inst.engine
                        nop.sync_info = mybir.SyncInfo(on_wait=[w], on_update=[])
                        newlist.append(nop)
                    inst.sync_info = mybir.SyncInfo(
                        on_wait=[list(si.on_wait)[-1]],
                        on_update=list(si.on_update))
                newlist.append(inst)
            live = bb.instructions
            live.clear()
            live.extend(newlist)
    return nc


def build_nc():
    """One-core SPMD program (all cores run this; data differs per core)."""
    nc = bass.Bass()

    hT_d = nc.declare_dram_parameter("hT", [128, NG, KC, GW], BF16, isOutput=False)
    wq_d = nc.declare_dram_parameter("wq", [128, KC, D], BF16, isOutput=False)
    wk_d = nc.declare_dram_parameter("wk", [128, KC, D], BF16, isOutput=False)
    wv_d = nc.declare_dram_parameter("wv", [128, KC, D], BF16, isOutput=False)
    aq_d = nc.declare_dram_parameter("Aq", [128, 2, S], BF16, isOutput=False)
    bq_d = nc.declare_dram_parameter("Bq", [128, 2, S], BF16, isOutput=False)
    ak_d = nc.declare_dram_parameter("Ak", [128, 2, S], BF16, isOutput=False)
    bk_d = nc.declare_dram_parameter("Bk", [128, 2, S], BF16, isOutput=False)
    wo_d = nc.declare_dram_parameter("wo", [128, 2, H], BF16, isOutput=False)
    mdT_d = nc.declare_dram_parameter("maskTd", [128, 128], BF16, isOutput=False)
    mpT_d = nc.declare_dram_parameter("maskTp", [128, 128], BF16, isOutput=False)
    out_d = nc.declare_dram_parameter("out", [S, H], BF16, isOutput=True)

    with tile.TileContext(nc) as tc:
        with (
            tc.tile_pool(name="persist", bufs=1) as persist,
            tc.tile_pool(name="ps512", bufs=3, space="PSUM") as ps512,
            tc.tile_pool(name="ps257", bufs=2, space="PSUM") as ps257,
        ):
            # persistent tensors
            v_aug = persist.tile([128, RB, D + 1], BF16, tag="vaug")
            qTf = persist.tile([128, 2, S], BF16, tag="qTf")   # roped+scaled
            kTf = persist.tile([128, 2, S], BF16, tag="kTf")   # roped, unscaled
            maskTd = persist.tile([128, 128], BF16, tag="mdT")
            maskTp = persist.tile([128, 128], BF16, tag="mpT")
            ident = persist.tile([128, 128], BF16, tag="ident")
            ones128 = persist.tile([128, 128], BF16, tag="ones128")

            wq_sb = persist.tile([128, KC, D], BF16, tag="wq")
            wk_sb = persist.tile([128, KC, D], BF16, tag="wk")
            wv_sb = persist.tile([128, KC, D], BF16, tag="wv")
            wo_sb = persist.tile([128, 2, H], BF16, tag="wo")

            # ---- DMA issue order: first-matmul inputs first ----
            for c0, c1 in ((0, 2), (2, 4)):
                nc.sync.dma_start(out=wq_sb[:, c0:c1, :],
                                  in_=wq_d[:, c0:c1, :])
            for dq in range(1, 5):
                nc.sync.dma_start(out=wq_sb[:, dq * 4:(dq + 1) * 4, :],
                                  in_=wq_d[:, dq * 4:(dq + 1) * 4, :])

            make_identity(nc, ident)
            # keep the PE HAM window busy while the first inputs stream in,
            # so the first real matmuls run at the warm clock
            with tc.tile_pool(name="warm", bufs=1, space="PSUM") as warmp:
                wps = warmp.tile([128, 128], F32, tag="warm")
                for _ in range(180):
                    nc.tensor.matmul(wps, lhsT=ident, rhs=ident,
                                     start=True, stop=True)

            nc.vector.memset(ones128, 1.0)
            nc.vector.memset(v_aug[:, :, D:D + 1], 1.0)
            eps_q = persist.tile([128, 1], F32, tag="eps_q")
            eps_k = persist.tile([128, 1], F32, tag="eps_k")
            nc.vector.memset(eps_q, EPS / (SCALING * SCALING))
            nc.vector.memset(eps_k, EPS)

            with (
                tc.tile_pool(name="ht", bufs=2) as htpool,
                tc.tile_pool(name="raw", bufs=2) as rawpool,
                tc.tile_pool(name="sq", bufs=2) as sqpool,
                tc.tile_pool(name="tabs", bufs=2) as tabpool,
                tc.tile_pool(name="rst", bufs=2) as rstpool,
                tc.tile_pool(name="tmp", bufs=4) as tmppool,
            ):
                for g in range(NG):
                    gsl = slice(g * GW, (g + 1) * GW)
                    ht = htpool.tile([128, KC, GW], BF16, tag="ht")
                    if g == 0:
                        for c0, c1 in ((0, 2), (2, 4)):
                            nc.sync.dma_start(out=ht[:, c0:c1, :],
                                              in_=hT_d[:, g, c0:c1, :])
                        chunks = range(1, 5)
                    else:
                        chunks = range(5)
                    for dq in chunks:
                        nc.sync.dma_start(
                            out=ht[:, dq * 4:(dq + 1) * 4, :],
                            in_=hT_d[:, g, dq * 4:(dq + 1) * 4, :],
                        )
                    if g == 0:
                        nc.sync.dma_start(out=wk_sb, in_=wk_d[:])
                        nc.sync.dma_start(out=wv_sb, in_=wv_d[:])
                        nc.sync.dma_start(out=maskTd, in_=mdT_d[:])
                        nc.sync.dma_start(out=maskTp, in_=mpT_d[:])
                    if g == 1:
                        for dc in range(2):
                            nc.sync.dma_start(out=wo_sb[:, dc, :],
                                              in_=wo_d[:, dc, :])

                    # rope tables for this group (streamed)
                    # ---- projection: q^T, k^T ----
                    qTr = rawpool.tile([128, 2, GW], BF16, tag="qTr")
                    kTr = rawpool.tile([128, 2, GW], BF16, tag="kTr")
                    for ti, (w_sb, outT) in enumerate(
                        ((wq_sb, qTr), (wk_sb, kTr))
                    ):
                        for dc in range(2):
                            ps = ps512.tile([128, GW], F32, tag="ps512")
                            for kc in range(KC):
                                nc.tensor.matmul(
                                    ps,
                                    lhsT=w_sb[:, kc, dc * 128:(dc + 1) * 128],
                                    rhs=ht[:, kc, :],
                                    start=(kc == 0), stop=(kc == KC - 1),
                                )
                            if (ti * 2 + dc) % 2 == 0:
                                nc.scalar.copy(outT[:, dc, :], ps)
                            else:
                                nc.vector.tensor_copy(outT[:, dc, :], ps)

                    # ---- projection: v natural, with ones column ----
                    for rbg in range(GW // 128):
                        rb = g * (GW // 128) + rbg
                        psv = ps257.tile([128, D + 1], F32, tag="ps257")
                        for kc in range(KC):
                            nc.tensor.matmul(
                                psv[:, 0:D],
                                lhsT=ht[:, kc, rbg * 128:(rbg + 1) * 128],
                                rhs=wv_sb[:, kc, :],
                                start=(kc == 0), stop=(kc == KC - 1),
                            )
                        nc.vector.tensor_copy(v_aug[:, rb, 0:D], psv[:, 0:D])

                    # rope tables for this group (after the proj DMAs
                    # so the first-matmul inputs win the DMA queues)
                    tA_q = tabpool.tile([128, 2, GW], BF16, tag="Aq")
                    tB_q = tabpool.tile([128, 2, GW], BF16, tag="Bq")
                    tA_k = tabpool.tile([128, 2, GW], BF16, tag="Ak")
                    tB_k = tabpool.tile([128, 2, GW], BF16, tag="Bk")
                    for t, d in ((tA_q, aq_d), (tB_q, bq_d),
                                 (tA_k, ak_d), (tB_k, bk_d)):
                        nc.sync.dma_start(out=t, in_=d[:, :, gsl])

                    # ---- squares (ACT) ----
                    sq_q = sqpool.tile([128, 2, GW], BF16, tag="sqq")
                    sq_k = sqpool.tile([128, 2, GW], BF16, tag="sqk")
                    for dc in range(2):
                        nc.scalar.square(sq_q[:, dc, :], qTr[:, dc, :])
                        nc.scalar.square(sq_k[:, dc, :], kTr[:, dc, :])

                    # ---- q stats: broadcast sum via ones-matmul ----
                    psq = ps512.tile([128, GW], F32, tag="ps512")
                    for dc in range(2):
                        nc.tensor.matmul(
                            psq, lhsT=ones128, rhs=sq_q[:, dc, :],
                            start=(dc == 0), stop=(dc == 1),
                        )
                    # rstd_q*SCALING = 1/sqrt((mean+eps)/SCALING^2)
                    rq = rstpool.tile([128, GW], F32, tag="rq")
                    nc.scalar.activation(
                        out=rq, in_=psq,
                        func=mybir.ActivationFunctionType.Sqrt,
                        scale=1.0 / (D * SCALING * SCALING),
                        bias=eps_q,
                    )
                    rqb = rstpool.tile([128, GW], BF16, tag="rqb")
                    with nc.allow_low_precision(
                            reason="bf16 rstd_q feeds a bf16 rope multiply"):
                        nc.vector.reciprocal(rqb, rq)

                    # ---- k stats: broadcast sum via ones-matmul ----
                    psk = ps512.tile([128, GW], F32, tag="ps512")
                    for dc in range(2):
                        nc.tensor.matmul(
                            psk, lhsT=ones128, rhs=sq_k[:, dc, :],
                            start=(dc == 0), stop=(dc == 1),
                        )
                    rk = rstpool.tile([128, GW], F32, tag="rk")
                    nc.scalar.activation(
                        out=rk, in_=psk,
                        func=mybir.ActivationFunctionType.Sqrt,
                        scale=1.0 / D, bias=eps_k,
                    )
                    rkb = rstpool.tile([128, GW], BF16, tag="rkb")
                    with nc.allow_low_precision(
                            reason="bf16 rstd_k feeds a bf16 rope multiply"):
                        nc.vector.reciprocal(rkb, rk)

                    # ---- rope: fin[dc] = raw[dc]*A[dc] + raw[1-dc]*B[dc] ----
                    for dc in range(2):
                        t1 = tmppool.tile([128, GW], BF16, tag="t1")
                        t2 = tmppool.tile([128, GW], BF16, tag="t2")
                        nc.vector.tensor_mul(t1, qTr[:, dc, :], tA_q[:, dc, :])
                        nc.vector.tensor_mul(t2, qTr[:, 1 - dc, :], tB_q[:, dc, :])
                        nc.vector.tensor_add(t1, t1, t2)
                        nc.vector.tensor_mul(qTf[:, dc, gsl], t1, rqb)
                        t3 = tmppool.tile([128, GW], BF16, tag="t1")
                        t4 = tmppool.tile([128, GW], BF16, tag="t2")
                        nc.vector.tensor_mul(t3, kTr[:, dc, :], tA_k[:, dc, :])
                        nc.vector.tensor_mul(t4, kTr[:, 1 - dc, :], tB_k[:, dc, :])
                        nc.vector.tensor_add(t3, t3, t4)
                        nc.vector.tensor_mul(kTf[:, dc, gsl], t3, rkb)

            # ------- phase B: attention + o_proj, merged per row block -------
            with (
                tc.tile_pool(name="probs", bufs=6) as prpool,
                tc.tile_pool(name="asmall", bufs=4) as aspool,
                tc.tile_pool(name="attnT", bufs=2) as atpool,
                tc.tile_pool(name="opool", bufs=2) as opool,
                tc.tile_pool(name="po", bufs=2, space="PSUM") as po,
                tc.tile_pool(name="psb", bufs=1, space="PSUM") as psb,
            ):
                for r in range(RB):
                    cmin = max(0, r - BLK_WIN)
                    cs = list(range(cmin, r + 1))
                    ps_at = ps257.tile([128, D + 1], F32, tag="ps257")
                    chunks = [cs[i0:i0 + 4] for i0 in range(0, len(cs), 4)]
                    # all score matmuls first: exps overlap later chunks'
                    # scores, so the attn@v accumulation never stalls PE
                    psWs = []
                    for chunk in chunks:
                        psW = ps512.tile([128, 512], F32, tag="ps512")
                        for j, c in enumerate(chunk):
                            sl = slice(j * 128, (j + 1) * 128)
                            masked = (c == r) or (c == r - BLK_WIN)
                            for dc in range(2):
                                nc.tensor.matmul(
                                    psW[:, sl],
                                    lhsT=kTf[:, dc, c * 128:(c + 1) * 128],
                                    rhs=qTf[:, dc, r * 128:(r + 1) * 128],
                                    start=(dc == 0),
                                    stop=(dc == 1) and not masked,
                                )
                            if masked:
                                nc.tensor.matmul(
                                    psW[:, sl],
                                    lhsT=maskTd if c == r else maskTp,
                                    rhs=ident,
                                    start=False, stop=True,
                                )
                        psWs.append(psW)
                    pTs = []
                    for chunk, psW in zip(chunks, psWs):
                        pT = prpool.tile([128, 512], BF16, tag="pT")
                        w = len(chunk) * 128
                        nc.scalar.activation(
                            out=pT[:, 0:w], in_=psW[:, 0:w],
                            func=mybir.ActivationFunctionType.Exp,
                        )
                        pTs.append(pT)
                    for chunk, pT in zip(chunks, pTs):
                        for j, c in enumerate(chunk):
                            sl = slice(j * 128, (j + 1) * 128)
                            nc.tensor.matmul(
                                ps_at,
                                lhsT=pT[:, sl],
                                rhs=v_aug[:, c, :],
                                start=(c == cmin), stop=(c == r),
                            )
                    rc = aspool.tile([128, 1], F32, tag="rc")
                    nc.vector.reciprocal(rc, ps_at[:, D:D + 1])
                    a_sb = aspool.tile([128, D], BF16, tag="asb")
                    nc.scalar.copy(a_sb, ps_at[:, 0:D])
                    attnT = atpool.tile([128, 2, 128], BF16, tag="attnT")
                    for dc in range(2):
                        pt = psb.tile([128, 128], BF16, tag="psb")
                        nc.tensor.transpose(
                            pt, a_sb[:, dc * 128:(dc + 1) * 128], ident
                        )
                        nc.vector.tensor_copy(attnT[:, dc, :], pt)

                    # o_proj for this row; denom recip folded into the drain
                    o_sb = opool.tile([128, H], BF16, tag="osb")
                    for hc in range(H // 512):
                        ps = po.tile([128, 512], F32, tag="po")
                        for dc in range(2):
                            nc.tensor.matmul(
                                ps,
                                lhsT=attnT[:, dc, :],
                                rhs=wo_sb[:, dc, hc * 512:(hc + 1) * 512],
                                start=(dc == 0), stop=(dc == 1),
                            )
                        if hc % 2 == 0:
                            nc.scalar.mul(o_sb[:, hc * 512:(hc + 1) * 512],
                                          ps, rc)
                        else:
                            with nc.allow_low_precision(
                                    reason="bf16 out slice; host sums in f32"):
                                nc.vector.tensor_scalar_mul(
                                    o_sb[:, hc * 512:(hc + 1) * 512], ps, rc)
                    nc.sync.dma_start(
                        out=out_d[r * 128:(r + 1) * 128, 0:1536],
                        in_=o_sb[:, 0:1536])
                    nc.sync.dma_start(
                        out=out_d[r * 128:(r + 1) * 128, 1536:H],
                        in_=o_sb[:, 1536:H])

    return nc


def make_in_maps(hidden_states, cos, sin, w_qkv, w_o, q_norm_w, k_norm_w):
    """Host-side sharding / relayout: one input map per core."""
    f32 = np.float32
    hT = _chunk_part(np.ascontiguousarray(hidden_states.T).astype(f32))
    hT = _bf16(np.ascontiguousarray(
        hT.reshape(128, KC, NG, GW).transpose(0, 2, 1, 3)))

    cosT = np.ascontiguousarray(cos.T).astype(f32)   # [D, S]
    sinT = np.ascontiguousarray(sin.T).astype(f32)

    def rope_tables(w):
        w1 = 1.0 + w.astype(f32)
        A = cosT * w1[:, None]
        B = np.empty_like(sinT)
        B[:128] = -sinT[:128] * w1[128:, None]
        B[128:] = sinT[128:] * w1[:128, None]
        return _bf16(_chunk_part(A)), _bf16(_chunk_part(B))

    Aq, Bq = rope_tables(q_norm_w)
    Ak, Bk = rope_tables(k_norm_w)

    jj = np.arange(128)[:, None]  # key index within block (partition)
    ii = np.arange(128)[None, :]  # query index within block (free)
    mask_diag = np.where(jj <= ii, 0.0, MASK_VAL).astype(f32)
    mask_part = np.where(jj >= ii + 1, 0.0, MASK_VAL).astype(f32)
    maskTd = _bf16(np.ascontiguousarray(mask_diag.T))
    maskTp = _bf16(np.ascontiguousarray(mask_part.T))

    in_maps = []
    for h in range(N_CORES):
        g = h // (NH // NKV)
        wq = _bf16(_chunk_part(np.ascontiguousarray(
            w_qkv[:, h * D:(h + 1) * D]).astype(f32)))
        wk = _bf16(_chunk_part(np.ascontiguousarray(
            w_qkv[:, NH * D + g * D: NH * D + (g + 1) * D]).astype(f32)))
        wv = _bf16(_chunk_part(np.ascontiguousarray(
            w_qkv[:, (NH + NKV) * D + g * D: (NH + NKV) * D + (g + 1) * D]
        ).astype(f32)))
        wo = _bf16(_chunk_part(np.ascontiguousarray(
            w_o[h * D:(h + 1) * D, :]).astype(f32)))
        in_maps.append({
            "hT": hT, "wq": wq, "wk": wk, "wv": wv,
            "Aq": Aq, "Bq": Bq, "Ak": Ak, "Bk": Bk,
            "wo": wo, "maskTd": maskTd, "maskTp": maskTp,
        })
    return in_maps


_NC_CACHE = None


def _get_nc():
    global _NC_CACHE
    if _NC_CACHE is None:
        _NC_CACHE = split_multiwaits(build_nc())
    return _NC_CACHE


def run(inputs, trace=False, **kw):
    """Returns (full_output, BassKernelResults)."""
    nc = _get_nc()
    in_maps = make_in_maps(**inputs)
    res = run_bass_kernel_spmd(
        nc, in_maps, core_ids=list(range(N_CORES)), trace=trace, **kw
    )
    parts = [np.asarray(res.results[i]["out"], dtype=np.float32)
             for i in range(N_CORES)]
    out = np.sum(np.stack(parts, axis=0), axis=0, dtype=np.float32)
    return out, res


def kernel(**inputs) -> np.ndarray:
    out, _ = run(inputs, trace=False)
    return out


# revision 16
# speedup vs baseline: 1.2118x; 1.2118x over previous
"""Gemma3 sliding-window attention layer on 8 Trainium2 NeuronCores.

Tensor-parallel over query heads: core h computes query head h (kv head
h//2), i.e. column-parallel qkv projection, full per-head attention, and
the row-parallel slice of o_proj; the 8 partial [S, H] bf16 outputs are
summed on the host in f32 (the all-reduce / unshard step).

Layout: hidden_states transposed on the host ([H, S]) so q and k come
out of the projection directly in [d, seq] layout (what the scores
matmul needs) and v comes out natural [seq, d] (what attn@v needs).
The softmax denominator rides as a ones-column appended to v.

Phase structure (interleaved per 512-token column group so Vector work
hides under the projection matmuls):
  per group g: qkv projection -> squares -> stats -> rope q/k
  per row r:   scores (+mask via PE accumulation) -> exp (k-rstd folded
               into the activation scale) -> attn@v -> transpose ->
               o_proj (softmax denom folded into the PSUM-drain scale)

RMSNorm: q's reciprocal-RMS is produced broadcast on all partitions by
a ones-matmul (with SCALING and the (1+w) gain folded into rope tables
/ eps); k's is produced per-key-partition (32 tiny matmuls) and applied
as the per-partition `scale` operand of the Exp activation, so k needs
no rstd multiply and no wide reciprocal at all.
"""

import numpy as np
import ml_dtypes

import concourse.bass as bass
import concourse.mybir as mybir
import concourse.tile as tile
from concourse.bass_utils import run_bass_kernel_spmd
from concourse.masks import make_identity

# ---- problem constants (hardcoded; kernel.py must be self-contained) ----
S = 2048          # sequence length
H = 2560          # hidden size
NH = 8            # query heads
NKV = 4           # kv heads
D = 256           # head dim
EPS = 1e-6
SCALING = 256.0 ** -0.5
WINDOW = 1024 - 1  # sliding window - 1

N_CORES = 8
KC = H // 128      # 20 contraction chunks for the projection
RB = S // 128      # 16 row blocks
GW = 512           # column group width
NG = S // GW       # 4 groups
MASK_VAL = -1e10
BLK_WIN = WINDOW // 128 + 1   # 8: c in [r-8, r] can contribute

F32 = mybir.dt.float32
BF16 = mybir.dt.bfloat16


def _bf16(x):
    return np.ascontiguousarray(x.astype(ml_dtypes.bfloat16))


def _chunk_part(x, p=128):
    """[c*p, n] -> [p, c, n] host relayout so it DMAs 1:1 into an SBUF tile."""
    c = x.shape[0] // p
    return np.ascontiguousarray(
        x.reshape(c, p, *x.shape[1:]).transpose(1, 0, *range(2, x.ndim + 1))
    )


def split_multiwaits(nc):
    """This toolchain's codegen allows one sync-wait slot per instruction.

    Tile emits several waits on the first consumer of multi-queue DMAs and
    on kernel-tail drains; hoist all but the last wait onto same-engine
    NoOps inserted immediately before the offending instruction (queue
    order on the engine preserves the semantics exactly).
    """
    k = 0
    for f in nc.m.functions:
        for bb in f.blocks:
            insts = bb.instructions
            if not any(i.sync_info and len(i.sync_info.on_wait) > 1
                       for i in insts):
                continue
            newlist = []
            for inst in insts:
                si = inst.sync_info
                if si is not None and len(si.on_wait) > 1:
                    for w in list(si.on_wait)[:-1]:
                        nop = mybir.InstNoOp(name=f"{inst.name}-ws{k}")
                        k += 1
                        nop.engine = inst.engine
                        nop.sync_info = mybir.SyncInfo(on_wait=[w], on_update=[])
                        newlist.append(nop)
                    inst.sync_info = mybir.SyncInfo(
                        on_wait=[list(si.on_wait)[-1]],
                        on_update=list(si.on_update))
                newlist.append(inst)
            live = bb.instructions
            live.clear()
            live.extend(newlist)
    return nc


def build_nc():
    """One-core SPMD program (all cores run this; data differs per core)."""
    nc = bass.Bass()

    hT_d = nc.declare_dram_parameter("hT", [128, NG, KC, GW], BF16, isOutput=False)
    wq_d = nc.declare_dram_parameter("wq", [128, KC, D], BF16, isOutput=False)
    wk_d = nc.declare_dram_parameter("wk", [128, KC, D], BF16, isOutput=False)
    wv_d = nc.declare_dram_parameter("wv", [128, KC, D], BF16, isOutput=False)
    aq_d = nc.declare_dram_parameter("Aq", [128, 2, S], BF16, isOutput=False)
    bq_d = nc.declare_dram_parameter("Bq", [128, 2, S], BF16, isOutput=False)
    ak_d = nc.declare_dram_parameter("Ak", [128, 2, S], BF16, isOutput=False)
    bk_d = nc.declare_dram_parameter("Bk", [128, 2, S], BF16, isOutput=False)
    wo_d = nc.declare_dram_parameter("wo", [128, 2, H], BF16, isOutput=False)
    mdT_d = nc.declare_dram_parameter("maskTd", [128, 128], BF16, isOutput=False)
    mpT_d = nc.declare_dram_parameter("maskTp", [128, 128], BF16, isOutput=False)
    out_d = nc.declare_dram_parameter("out", [S, H], BF16, isOutput=True)

    with tile.TileContext(nc) as tc:
        with (
            tc.tile_pool(name="persist", bufs=1) as persist,
            tc.tile_pool(name="ps512", bufs=3, space="PSUM") as ps512,
            tc.tile_pool(name="ps257", bufs=2, space="PSUM") as ps257,
        ):
            # persistent tensors
            v_aug = persist.tile([128, RB, D + 1], BF16, tag="vaug")
            qTf = persist.tile([128, 2, S], BF16, tag="qTf")   # roped+scaled
            kTf = persist.tile([128, 2, S], BF16, tag="kTf")   # roped, unscaled
            maskTd = persist.tile([128, 128], BF16, tag="mdT")
            maskTp = persist.tile([128, 128], BF16, tag="mpT")
            ident = persist.tile([128, 128], BF16, tag="ident")
            ones128 = persist.tile([128, 128], BF16, tag="ones128")

            wq_sb = persist.tile([128, KC, D], BF16, tag="wq")
            wk_sb = persist.tile([128, KC, D], BF16, tag="wk")
            wv_sb = persist.tile([128, KC, D], BF16, tag="wv")
            wo_sb = persist.tile([128, 2, H], BF16, tag="wo")

            # ---- DMA issue order: first-matmul inputs first ----
            for c0, c1 in ((0, 2), (2, 4)):
                nc.sync.dma_start(out=wq_sb[:, c0:c1, :],
                                  in_=wq_d[:, c0:c1, :])
            for dq in range(1, 5):
                nc.sync.dma_start(out=wq_sb[:, dq * 4:(dq + 1) * 4, :],
                                  in_=wq_d[:, dq * 4:(dq + 1) * 4, :])

            make_identity(nc, ident)
            nc.vector.memset(ones128, 1.0)
            nc.vector.memset(v_aug[:, :, D:D + 1], 1.0)
            eps_q = persist.tile([128, 1], F32, tag="eps_q")
            eps_k = persist.tile([128, 1], F32, tag="eps_k")
            nc.vector.memset(eps_q, EPS / (SCALING * SCALING))
            nc.vector.memset(eps_k, EPS)

            with (
                tc.tile_pool(name="ht", bufs=2) as htpool,
                tc.tile_pool(name="raw", bufs=2) as rawpool,
                tc.tile_pool(name="sq", bufs=2) as sqpool,
                tc.tile_pool(name="tabs", bufs=2) as tabpool,
                tc.tile_pool(name="rst", bufs=2) as rstpool,
                tc.tile_pool(name="tmp", bufs=4) as tmppool,
            ):
                for g in range(NG):
                    gsl = slice(g * GW, (g + 1) * GW)
                    ht = htpool.tile([128, KC, GW], BF16, tag="ht")
                    if g == 0:
                        for c0, c1 in ((0, 2), (2, 4)):
                            nc.sync.dma_start(out=ht[:, c0:c1, :],
                                              in_=hT_d[:, g, c0:c1, :])
                        chunks = range(1, 5)
                    else:
                        chunks = range(5)
                    for dq in chunks:
                        nc.sync.dma_start(
                            out=ht[:, dq * 4:(dq + 1) * 4, :],
                            in_=hT_d[:, g, dq * 4:(dq + 1) * 4, :],
                        )
                    if g == 0:
                        nc.sync.dma_start(out=wk_sb, in_=wk_d[:])
                        nc.sync.dma_start(out=wv_sb, in_=wv_d[:])
                        nc.sync.dma_start(out=maskTd, in_=mdT_d[:])
                        nc.sync.dma_start(out=maskTp, in_=mpT_d[:])
                    if g == 1:
                        for dc in range(2):
                            nc.sync.dma_start(out=wo_sb[:, dc, :],
                                              in_=wo_d[:, dc, :])

                    # rope tables for this group (streamed)
                    # ---- projection: q^T, k^T ----
                    qTr = rawpool.tile([128, 2, GW], BF16, tag="qTr")
                    kTr = rawpool.tile([128, 2, GW], BF16, tag="kTr")
                    for ti, (w_sb, outT) in enumerate(
                        ((wq_sb, qTr), (wk_sb, kTr))
                    ):
                        for dc in range(2):
                            ps = ps512.tile([128, GW], F32, tag="ps512")
                            for kc in range(KC):
                                nc.tensor.matmul(
                                    ps,
                                    lhsT=w_sb[:, kc, dc * 128:(dc + 1) * 128],
                                    rhs=ht[:, kc, :],
                                    start=(kc == 0), stop=(kc == KC - 1),
                                )
                            if (ti * 2 + dc) % 2 == 0:
                                nc.scalar.copy(outT[:, dc, :], ps)
                            else:
                                nc.vector.tensor_copy(outT[:, dc, :], ps)

                    # ---- projection: v natural, with ones column ----
                    for rbg in range(GW // 128):
                        rb = g * (GW // 128) + rbg
                        psv = ps257.tile([128, D + 1], F32, tag="ps257")
                        for kc in range(KC):
                            nc.tensor.matmul(
                                psv[:, 0:D],
                                lhsT=ht[:, kc, rbg * 128:(rbg + 1) * 128],
                                rhs=wv_sb[:, kc, :],
                                start=(kc == 0), stop=(kc == KC - 1),
                            )
                        nc.vector.tensor_copy(v_aug[:, rb, 0:D], psv[:, 0:D])

                    # rope tables for this group (after the proj DMAs
                    # so the first-matmul inputs win the DMA queues)
                    tA_q = tabpool.tile([128, 2, GW], BF16, tag="Aq")
                    tB_q = tabpool.tile([128, 2, GW], BF16, tag="Bq")
                    tA_k = tabpool.tile([128, 2, GW], BF16, tag="Ak")
                    tB_k = tabpool.tile([128, 2, GW], BF16, tag="Bk")
                    for t, d in ((tA_q, aq_d), (tB_q, bq_d),
                                 (tA_k, ak_d), (tB_k, bk_d)):
                        nc.sync.dma_start(out=t, in_=d[:, :, gsl])

                    # ---- squares (ACT) ----
                    sq_q = sqpool.tile([128, 2, GW], BF16, tag="sqq")
                    sq_k = sqpool.tile([128, 2, GW], BF16, tag="sqk")
                    for dc in range(2):
                        nc.scalar.square(sq_q[:, dc, :], qTr[:, dc, :])
                        nc.scalar.square(sq_k[:, dc, :], kTr[:, dc, :])

                    # ---- q stats: broadcast sum via ones-matmul ----
                    psq = ps512.tile([128, GW], F32, tag="ps512")
                    for dc in range(2):
                        nc.tensor.matmul(
                            psq, lhsT=ones128, rhs=sq_q[:, dc, :],
                            start=(dc == 0), stop=(dc == 1),
                        )
                    # rstd_q*SCALING = 1/sqrt((mean+eps)/SCALING^2)
                    rq = rstpool.tile([128, GW], F32, tag="rq")
                    nc.scalar.activation(
                        out=rq, in_=psq,
                        func=mybir.ActivationFunctionType.Sqrt,
                        scale=1.0 / (D * SCALING * SCALING),
                        bias=eps_q,
                    )
                    rqb = rstpool.tile([128, GW], BF16, tag="rqb")
                    with nc.allow_low_precision(
                            reason="bf16 rstd_q feeds a bf16 rope multiply"):
                        nc.vector.reciprocal(rqb, rq)

                    # ---- k stats: broadcast sum via ones-matmul ----
                    psk = ps512.tile([128, GW], F32, tag="ps512")
                    for dc in range(2):
                        nc.tensor.matmul(
                            psk, lhsT=ones128, rhs=sq_k[:, dc, :],
                            start=(dc == 0), stop=(dc == 1),
                        )
                    rk = rstpool.tile([128, GW], F32, tag="rk")
                    nc.scalar.activation(
                        out=rk, in_=psk,
                        func=mybir.ActivationFunctionType.Sqrt,
                        scale=1.0 / D, bias=eps_k,
                    )
                    rkb = rstpool.tile([128, GW], BF16, tag="rkb")
                    with nc.allow_low_precision(
                            reason="bf16 rstd_k feeds a bf16 rope multiply"):
                        nc.vector.reciprocal(rkb, rk)

                    # ---- rope: fin[dc] = raw[dc]*A[dc] + raw[1-dc]*B[dc] ----
                    for dc in range(2):
                        t1 = tmppool.tile([128, GW], BF16, tag="t1")
                        t2 = tmppool.tile([128, GW], BF16, tag="t2")
                        nc.vector.tensor_mul(t1, qTr[:, dc, :], tA_q[:, dc, :])
                        nc.vector.tensor_mul(t2, qTr[:, 1 - dc, :], tB_q[:, dc, :])
                        nc.vector.tensor_add(t1, t1, t2)
                        nc.vector.tensor_mul(qTf[:, dc, gsl], t1, rqb)
                        t3 = tmppool.tile([128, GW], BF16, tag="t1")
                        t4 = tmppool.tile([128, GW], BF16, tag="t2")
                        nc.vector.tensor_mul(t3, kTr[:, dc, :], tA_k[:, dc, :])
                        nc.vector.tensor_mul(t4, kTr[:, 1 - dc, :], tB_k[:, dc, :])
                        nc.vector.tensor_add(t3, t3, t4)
                        nc.vector.tensor_mul(kTf[:, dc, gsl], t3, rkb)

            # ------- phase B: attention + o_proj, merged per row block -------
            with (
                tc.tile_pool(name="probs", bufs=6) as prpool,
                tc.tile_pool(name="asmall", bufs=4) as aspool,
                tc.tile_pool(name="attnT", bufs=2) as atpool,
                tc.tile_pool(name="opool", bufs=2) as opool,
                tc.tile_pool(name="po", bufs=2, space="PSUM") as po,
                tc.tile_pool(name="psb", bufs=1, space="PSUM") as psb,
            ):
                for r in range(RB):
                    cmin = max(0, r - BLK_WIN)
                    cs = list(range(cmin, r + 1))
                    ps_at = ps257.tile([128, D + 1], F32, tag="ps257")
                    chunks = [cs[i0:i0 + 4] for i0 in range(0, len(cs), 4)]
                    # all score matmuls first: exps overlap later chunks'
                    # scores, so the attn@v accumulation never stalls PE
                    psWs = []
                    for chunk in chunks:
                        psW = ps512.tile([128, 512], F32, tag="ps512")
                        for j, c in enumerate(chunk):
                            sl = slice(j * 128, (j + 1) * 128)
                            masked = (c == r) or (c == r - BLK_WIN)
                            for dc in range(2):
                                nc.tensor.matmul(
                                    psW[:, sl],
                                    lhsT=kTf[:, dc, c * 128:(c + 1) * 128],
                                    rhs=qTf[:, dc, r * 128:(r + 1) * 128],
                                    start=(dc == 0),
                                    stop=(dc == 1) and not masked,
                                )
                            if masked:
                                nc.tensor.matmul(
                                    psW[:, sl],
                                    lhsT=maskTd if c == r else maskTp,
                                    rhs=ident,
                                    start=False, stop=True,
                                )
                        psWs.append(psW)
                    pTs = []
                    for chunk, psW in zip(chunks, psWs):
                        pT = prpool.tile([128, 512], BF16, tag="pT")
                        w = len(chunk) * 128
                        nc.scalar.activation(
                            out=pT[:, 0:w], in_=psW[:, 0:w],
                            func=mybir.ActivationFunctionType.Exp,
                        )
                        pTs.append(pT)
                    for chunk, pT in zip(chunks, pTs):
                        for j, c in enumerate(chunk):
                            sl = slice(j * 128, (j + 1) * 128)
                            nc.tensor.matmul(
                                ps_at,
                                lhsT=pT[:, sl],
                                rhs=v_aug[:, c, :],
                                start=(c == cmin), stop=(c == r),
                            )
                    rc = aspool.tile([128, 1], F32, tag="rc")
                    nc.vector.reciprocal(rc, ps_at[:, D:D + 1])
                    a_sb = aspool.tile([128, D], BF16, tag="asb")
                    nc.scalar.copy(a_sb, ps_at[:, 0:D])
                    attnT = atpool.tile([128, 2, 128], BF16, tag="attnT")
                    for dc in range(2):
                        pt = psb.tile([128, 128], BF16, tag="psb")
                        nc.tensor.transpose(
                            pt, a_sb[:, dc * 128:(dc + 1) * 128], ident
                        )
                        nc.vector.tensor_copy(attnT[:, dc, :], pt)

                    # o_proj for this row; denom recip folded into the drain
                    o_sb = opool.tile([128, H], BF16, tag="osb")
                    for hc in range(H // 512):
                        ps = po.tile([128, 512], F32, tag="po")
                        for dc in range(2):
                            nc.tensor.matmul(
                                ps,
                                lhsT=attnT[:, dc, :],
                                rhs=wo_sb[:, dc, hc * 512:(hc + 1) * 512],
                                start=(dc == 0), stop=(dc == 1),
                            )
                        if hc % 2 == 0:
                            nc.scalar.mul(o_sb[:, hc * 512:(hc + 1) * 512],
                                          ps, rc)
                        else:
                            with nc.allow_low_precision(
                                    reason="bf16 out slice; host sums in f32"):
                                nc.vector.tensor_scalar_mul(
                                    o_sb[:, hc * 512:(hc + 1) * 512], ps, rc)
                    nc.sync.dma_start(
                        out=out_d[r * 128:(r + 1) * 128, 0:1536],
                        in_=o_sb[:, 0:1536])
                    nc.sync.dma_start(
                        out=out_d[r * 128:(r + 1) * 128, 1536:H],
                        in_=o_sb[:, 1536:H])

    return nc


def make_in_maps(hidden_states, cos, sin, w_qkv, w_o, q_norm_w, k_norm_w):
    """Host-side sharding / relayout: one input map per core."""
    f32 = np.float32
    hT = _chunk_part(np.ascontiguousarray(hidden_states.T).astype(f32))
    hT = _bf16(np.ascontiguousarray(
        hT.reshape(128, KC, NG, GW).transpose(0, 2, 1, 3)))

    cosT = np.ascontiguousarray(cos.T).astype(f32)   # [D, S]
    sinT = np.ascontiguousarray(sin.T).astype(f32)

    def rope_tables(w):
        w1 = 1.0 + w.astype(f32)
        A = cosT * w1[:, None]
        B = np.empty_like(sinT)
        B[:128] = -sinT[:128] * w1[128:, None]
        B[128:] = sinT[128:] * w1[:128, None]
        return _bf16(_chunk_part(A)), _bf16(_chunk_part(B))

    Aq, Bq = rope_tables(q_norm_w)
    Ak, Bk = rope_tables(k_norm_w)

    jj = np.arange(128)[:, None]  # key index within block (partition)
    ii = np.arange(128)[None, :]  # query index within block (free)
    mask_diag = np.where(jj <= ii, 0.0, MASK_VAL).astype(f32)
    mask_part = np.where(jj >= ii + 1, 0.0, MASK_VAL).astype(f32)
    maskTd = _bf16(np.ascontiguousarray(mask_diag.T))
    maskTp = _bf16(np.ascontiguousarray(mask_part.T))

    in_maps = []
    for h in range(N_CORES):
        g = h // (NH // NKV)
        wq = _bf16(_chunk_part(np.ascontiguousarray(
            w_qkv[:, h * D:(h + 1) * D]).astype(f32)))
        wk = _bf16(_chunk_part(np.ascontiguousarray(
            w_qkv[:, NH * D + g * D: NH * D + (g + 1) * D]).astype(f32)))
        wv = _bf16(_chunk_part(np.ascontiguousarray(
            w_qkv[:, (NH + NKV) * D + g * D: (NH + NKV) * D + (g + 1) * D]
        ).astype(f32)))
        wo = _bf16(_chunk_part(np.ascontiguousarray(
            w_o[h * D:(h + 1) * D, :]).astype(f32)))
        in_maps.append({
            "hT": hT, "wq": wq, "wk": wk, "wv": wv,
            "Aq": Aq, "Bq": Bq, "Ak": Ak, "Bk": Bk,
            "wo": wo, "maskTd": maskTd, "maskTp": maskTp,
        })
    return in_maps


_NC_CACHE = None


def _get_nc():
    global _NC_CACHE
    if _NC_CACHE is None:
        _NC_CACHE = split_multiwaits(build_nc())
    return _NC_CACHE


def run(inputs, trace=False, **kw):
    """Returns (full_output, BassKernelResults)."""
    nc = _get_nc()
    in_maps = make_in_maps(**inputs)
    res = run_bass_kernel_spmd(
        nc, in_maps, core_ids=list(range(N_CORES)), trace=trace, **kw
    )
    parts = [np.asarray(res.results[i]["out"], dtype=np.float32)
             for i in range(N_CORES)]
    out = np.sum(np.stack(parts, axis=0), axis=0, dtype=np.float32)
    return out, res


def kernel(**inputs) -> np.ndarray:
    out, _ = run(inputs, trace=False)
    return out
